# revision 34
# baseline (speedup 1.0000x reference)
"""Trainium2 Bass kernel for nn_DiffusionBlock: 20 steps of a 5-point
reflect-padded diffusion stencil on (16, 1, 1024, 1024) fp32.

The step operator is linear/separable and diagonalized analytically by the
DCT-I basis v_k[i] = cos(pi*k*i/(N-1)); the T-step result is the spectral
map Y = F [ M * (E^T X E) ] F^T with M_ij = (lv_i + lw_j)^T. Eigenvector
parity folds X into 4 parity quadrants (512x512). After 20 steps the mask
M decays to two spectral corners (|cos| near 1); 144 modes per corner per
axis keep max rel err at 1.1e-2 (emulation == hardware, deterministic), so
each quadrant pipeline runs on a compact 288-mode spectral axis zero-padded
to 384 (3 partition chunks of 128).

Everything is bf16 (same PE rate as fp32r, half the DMA/SBUF, and no
256-wide fp32r free-size floor), PSUM accumulation in fp32. Per quadrant:
  S1: W  = X^T E_r            [c-tile, 288]    16 mm x 288-free
  S2: Z' = (M^T) o (E_g^T W)  [k2-tile, k1]    12 mm, frees 144/288/144
  S4: C  = Z'^T Ft_g          [k1-tile, 512]    7 mm x 512 (corner-sparse)
  S5: Y  = C^T Ft_r           [i1-tile, 512]   12 mm x 512
Jobs (2 images x 4 quadrants) are software-pipelined (S2 lags S1 by one
job, S4 by three, S5 by four) so PSUM drains (Act for W/Y, DVE for the
Z' mask-multiply and C) hide under the matmul stream; matmul cost on TRN2
is output-free-size x 1 cycle regardless of dtype/contraction, so stages
are oriented to keep free dims at 288-512 and lhsT loads at 128x128.
Data-parallel over batch: 2 images/core on 8 NeuronCores.
"""

import sys

import ml_dtypes
import numpy as np

if "/opt/trn_rl_repo" not in sys.path:
    sys.path.insert(0, "/opt/trn_rl_repo")

import concourse.bass as bass  # noqa: E402
import concourse.tile as tile  # noqa: E402
from concourse import bacc, mybir  # noqa: E402
from concourse.bass_utils import run_bass_kernel_spmd  # noqa: E402

N = 1024
H = 512           # half size after parity fold
P = 128
WA = 144          # live modes per spectral corner (per axis, per quadrant)
LV = 2 * WA       # live modes per axis
K = 384           # padded spectral axis: 3 chunks of 128 (64 zero modes)
KC = K // P       # 3 chunks
NCORES = 8
IPC = 2           # images per core
BF16 = ml_dtypes.bfloat16

RHO = [0, 0, 1, 1]
GAM = [0, 1, 0, 1]
# S2 free (k1) range per k2-chunk: corner A needs [0,WA), the mixed middle
# chunk needs everything live, corner B needs [WA,LV)
S2_RANGE = [(0, WA), (0, LV), (WA, LV)]
# mask tile offsets within a quadrant's mq strip (widths 160/320/160)
MQ_OFF = [0, WA, WA + LV]
MQ_W = 2 * WA + LV  # per quadrant
# S4 live k2-chunks per k1-tile (corner-dead (0,2)/(2,0) blocks skipped)
S4_LIVE = [(0, 1), (0, 1, 2), (1, 2)]

_BASS_CACHE = {}
_MAT_CACHE = {}


def _chunk(a):
    """(512, w) -> (128, 4*w); chunk r holds rows [128r, 128r+128)."""
    w = a.shape[1]
    return np.ascontiguousarray(
        a.reshape(4, P, w).transpose(1, 0, 2).reshape(P, 4 * w))


def _chunkK(a):
    """(384, 512) -> (128, 3*512); chunk t holds rows [128t, 128t+128)."""
    return np.ascontiguousarray(
        a.reshape(KC, P, H).transpose(1, 0, 2).reshape(P, KC * H))


def _build_specs(weight, time_steps):
    key = (weight.tobytes(), int(time_steps))
    if key in _MAT_CACHE:
        return _MAT_CACHE[key]
    w = np.asarray(weight, dtype=np.float64).reshape(3, 3)
    assert max(abs(w[0, 0]), abs(w[0, 2]), abs(w[2, 0]), abs(w[2, 2])) < 1e-12
    assert abs(w[0, 1] - w[2, 1]) < 1e-12 and abs(w[1, 0] - w[1, 2]) < 1e-12
    a_c = w[1, 1]
    k = np.arange(N)
    i = np.arange(N)
    lam = np.cos(np.pi * k / (N - 1))
    V = np.cos(np.pi * np.outer(i, k) / (N - 1))
    d = np.ones(N)
    d[0] = 0.5
    d[-1] = 0.5
    wn = np.sqrt((d[:, None] * V * V).sum(axis=0))
    E = (d[:, None] * V) / wn[None, :]
    Fm = V / wn[None, :]
    lv = (w[0, 1] + w[2, 1]) * lam
    lw = (w[1, 0] + w[1, 2]) * lam
    M = (a_c + lv[:, None] + lw[None, :]) ** int(time_steps)

    halves = (np.arange(0, N, 2), np.arange(1, N, 2))
    live = np.r_[0:WA, H - WA:H]

    def pad(a):  # pad live axis LV -> K with zero modes
        out = np.zeros((a.shape[0], K), a.dtype)
        out[:, :LV] = a
        return out

    # eL[h]: E half-h, live columns (zero-padded), chunked: [128, 4*384]
    eL = np.concatenate(
        [_chunk(pad(E[:H, hv][:, live])) for hv in halves], axis=1)
    # ft[h]: F half-h live cols transposed [k,i], k-chunked: [128, 3*512]
    ft = np.concatenate(
        [_chunkK(np.ascontiguousarray(pad(Fm[:H, hv][:, live]).T))
         for hv in halves], axis=1)
    # mq: per quadrant, M^T on the compact axes, k2-chunked, k1-trimmed
    mqs = []
    for q in range(4):
        r_idx = halves[RHO[q]][live]
        g_idx = halves[GAM[q]][live]
        MqT = np.zeros((K, K), np.float64)  # [k2, k1], zero-padded
        MqT[:LV, :LV] = M[np.ix_(r_idx, g_idx)].T
        strips = []
        for t in range(KC):
            lo, hi = S2_RANGE[t]
            strips.append(MqT[t * P:(t + 1) * P, lo:hi])
        mqs.append(np.concatenate(strips, axis=1))
    mq = np.concatenate(mqs, axis=1)
    out = (eL.astype(BF16), ft.astype(BF16), mq.astype(BF16))
    _MAT_CACHE[key] = out
    return out


def _fold_image(img):
    """(1024, 1024) f32 -> (128, 8192) bf16 quadrant-folded chunk layout."""
    a = img.astype(np.float32)
    xp = a[:H] + a[N - 1:H - 1:-1]
    xm = a[:H] - a[N - 1:H - 1:-1]
    qs = []
    for xr in (xp, xm):
        qs.append(xr[:, :H] + xr[:, N - 1:H - 1:-1])
        qs.append(xr[:, :H] - xr[:, N - 1:H - 1:-1])
    return np.concatenate([_chunk(q) for q in qs], axis=1).astype(BF16)


def _unchunk(t):
    return np.ascontiguousarray(
        t.reshape(P, 4, H).transpose(1, 0, 2).reshape(H, H))


def _unfold_image(yq):
    """(128, 8192) bf16 quadrant outputs -> (1024, 1024) f32."""
    Qs = [_unchunk(yq[:, 2048 * q: 2048 * (q + 1)].astype(np.float64))
          for q in range(4)]
    Ypp, Ypm, Ymp, Ymm = Qs
    Y = np.empty((N, N), dtype=np.float32)
    Y[:H, :H] = Ypp + Ypm + Ymp + Ymm
    Y[:H, H:] = (Ypp - Ypm + Ymp - Ymm)[:, ::-1]
    Y[H:, :H] = (Ypp + Ypm - Ymp - Ymm)[::-1, :]
    Y[H:, H:] = (Ypp - Ypm - Ymp + Ymm)[::-1, ::-1]
    return Y


def _build_bass():
    if "nc" in _BASS_CACHE:
        return _BASS_CACHE["nc"]
    nc = bacc.Bacc("TRN2", target_bir_lowering=False, debug=False,
                   num_devices=NCORES)
    bf = mybir.dt.bfloat16
    f32 = mybir.dt.float32
    xq_d = nc.dram_tensor("xq", [IPC, P, 8192], bf, kind="ExternalInput").ap()
    eL_d = nc.dram_tensor("eL", [P, 2 * 4 * K], bf, kind="ExternalInput").ap()
    ft_d = nc.dram_tensor("ft", [P, 2 * KC * H], bf,
                          kind="ExternalInput").ap()
    mq_d = nc.dram_tensor("mq", [P, 4 * MQ_W], bf, kind="ExternalInput").ap()
    yq_d = nc.dram_tensor("yq", [IPC, P, 8192], bf, kind="ExternalOutput").ap()

    with tile.TileContext(nc) as tc:
        with tc.tile_pool(name="const", bufs=1) as cpool, \
             tc.tile_pool(name="data", bufs=4) as dpool, \
             tc.tile_pool(name="ydat", bufs=12) as ypool, \
             tc.tile_pool(name="psum", bufs=8, space="PSUM") as ppool:
            eL_t = cpool.tile([P, 2 * 4 * K], bf, tag="eL")
            ft_t = cpool.tile([P, 2 * KC * H], bf, tag="ft")
            mq_t = cpool.tile([P, 4 * MQ_W], bf, tag="mq")
            xq_t = [cpool.tile([P, 8192], bf, tag=f"xq{i}", name=f"xq{i}")
                    for i in range(IPC)]
            # Z' tiles: 3 pipeline sets x 3 k2-chunks; the never-written k1
            # ranges ([WA,K) of t0, [0,WA) of t2) must read as zero in S4
            zt_t = [[cpool.tile([P, K], bf, tag=f"zt{s}_{t}", name=f"zt{s}_{t}")
                     for t in range(KC)] for s in range(6)]
            for s in range(6):
                nc.gpsimd.memset(zt_t[s][0][:, WA:K], 0.0)
                nc.gpsimd.memset(zt_t[s][1][:, LV:K], 0.0)
                nc.gpsimd.memset(zt_t[s][2][:, 0:WA], 0.0)
                nc.gpsimd.memset(zt_t[s][2][:, LV:K], 0.0)

            # input DMA, fine-grained first-use order: S1 job0 is rc-outer,
            # so round rc needs only xq chunk rc + eL chunk rc
            def dx(i, c0, c1):
                s = slice(512 * c0, 512 * c1)
                nc.sync.dma_start(out=xq_t[i][:, s], in_=xq_d[i, :, s])

            for rc in range(4):
                nc.sync.dma_start(out=eL_t[:, K * rc:K * (rc + 1)],
                                  in_=eL_d[:, K * rc:K * (rc + 1)])
                dx(0, rc, rc + 1)
            nc.sync.dma_start(out=mq_t[:, 0:MQ_W], in_=mq_d[:, 0:MQ_W])
            dx(0, 4, 8)                                       # xq0 q1
            nc.sync.dma_start(out=ft_t[:, 0:KC * H], in_=ft_d[:, 0:KC * H])
            nc.sync.dma_start(out=mq_t[:, MQ_W:2 * MQ_W],
                              in_=mq_d[:, MQ_W:2 * MQ_W])
            nc.sync.dma_start(out=eL_t[:, 4 * K:8 * K],
                              in_=eL_d[:, 4 * K:8 * K])
            dx(0, 8, 12)                                      # xq0 q2
            nc.sync.dma_start(out=ft_t[:, KC * H:2 * KC * H],
                              in_=ft_d[:, KC * H:2 * KC * H])
            nc.sync.dma_start(out=mq_t[:, 2 * MQ_W:4 * MQ_W],
                              in_=mq_d[:, 2 * MQ_W:4 * MQ_W])
            dx(0, 12, 16)                                     # xq0 q3
            for c in range(0, 16, 4):
                dx(1, c, c + 4)

            jobs = [(img, q) for img in range(IPC) for q in range(4)]
            nj = len(jobs)
            state = {}

            def s1(j):
                img, q = jobs[j]
                rho = RHO[q]
                w_t = dpool.tile([P, 4 * K], bf, tag="w")
                for ct in range(4):
                    ps = ppool.tile([P, K], f32, tag="ps")
                    for rc in range(4):
                        nc.tensor.matmul(
                            out=ps[:, 0:LV],
                            lhsT=xq_t[img][:, 2048 * q + 512 * rc
                                           + P * ct: 2048 * q + 512 * rc
                                           + P * (ct + 1)],
                            rhs=eL_t[:, 4 * K * rho + K * rc:
                                     4 * K * rho + K * rc + LV],
                            start=(rc == 0), stop=(rc == 3))
                    nc.scalar.copy(out=w_t[:, K * ct:K * ct + LV],
                                   in_=ps[:, 0:LV])
                state[("w", j)] = w_t

            def s2(j):
                img, q = jobs[j]
                gam = GAM[q]
                w_t = state.pop(("w", j))
                zts = zt_t[j % 6]
                for t in range(KC):
                    lo, hi = S2_RANGE[t]
                    ps = ppool.tile([P, K], f32, tag="ps")
                    for cc in range(4):
                        nc.tensor.matmul(
                            out=ps[:, lo:hi],
                            lhsT=eL_t[:, 4 * K * gam + K * cc + P * t:
                                      4 * K * gam + K * cc + P * (t + 1)],
                            rhs=w_t[:, K * cc + lo:K * cc + hi],
                            start=(cc == 0), stop=(cc == 3))
                    nc.vector.tensor_tensor(
                        out=zts[t][:, lo:hi], in0=ps[:, lo:hi],
                        in1=mq_t[:, MQ_W * q + MQ_OFF[t]:
                                 MQ_W * q + MQ_OFF[t] + (hi - lo)],
                        op=mybir.AluOpType.mult)

            def s4(j):
                img, q = jobs[j]
                gam = GAM[q]
                zts = zt_t[j % 6]
                c_t = dpool.tile([P, KC * H], bf, tag="c")
                for b in range(KC):
                    ps = ppool.tile([P, H], f32, tag="ps")
                    lv = S4_LIVE[b]
                    for t in lv:
                        nc.tensor.matmul(
                            out=ps[:, 0:H],
                            lhsT=zts[t][:, P * b:P * (b + 1)],
                            rhs=ft_t[:, KC * H * gam + H * t:
                                     KC * H * gam + H * (t + 1)],
                            start=(t == lv[0]), stop=(t == lv[-1]))
                    nc.vector.tensor_copy(out=c_t[:, H * b:H * (b + 1)],
                                          in_=ps[:, 0:H])
                state[("c", j)] = c_t

            def s5(j):
                img, q = jobs[j]
                rho = RHO[q]
                c_t = state.pop(("c", j))
                for it in range(4):
                    ps = ppool.tile([P, H], f32, tag="ps")
                    for kc in range(KC):
                        nc.tensor.matmul(
                            out=ps[:, 0:H],
                            lhsT=ft_t[:, KC * H * rho + H * kc + P * it:
                                      KC * H * rho + H * kc + P * (it + 1)],
                            rhs=c_t[:, H * kc:H * (kc + 1)],
                            start=(kc == 0), stop=(kc == KC - 1))
                    y_t = ypool.tile([P, H], bf, tag="y")
                    yo = 2048 * q + H * it
                    # final job: split drains across Act/DVE to cut the tail
                    if j == nj - 1 and it % 2:
                        nc.vector.tensor_copy(out=y_t[:, 0:H], in_=ps[:, 0:H])
                        dma_eng = nc.gpsimd
                    else:
                        nc.scalar.copy(out=y_t[:, 0:H], in_=ps[:, 0:H])
                        dma_eng = nc.scalar
                    dma_eng.dma_start(out=yq_d[img, :, yo:yo + H],
                                      in_=y_t[:, 0:H])

            for step in range(nj + 4):
                if step < nj:
                    s1(step)
                if 1 <= step < nj + 1:
                    s2(step - 1)
                if 3 <= step < nj + 3:
                    s4(step - 3)
                if 4 <= step:
                    s5(step - 4)

    nc.compile()
    _BASS_CACHE["nc"] = nc
    return nc


def kernel(x, weight, time_steps, **_ignored):
    x = np.asarray(x, dtype=np.float32)
    weight = np.asarray(weight, dtype=np.float32)
    eL, ft, mq = _build_specs(weight, time_steps)
    nc = _build_bass()

    b = x.shape[0]
    assert b == NCORES * IPC and x.shape[-2:] == (N, N)
    in_maps = []
    for c in range(NCORES):
        xq = np.stack([_fold_image(x[c * IPC + i, 0]) for i in range(IPC)])
        in_maps.append({"xq": xq, "eL": eL, "ft": ft, "mq": mq})

    res = run_bass_kernel_spmd(nc, in_maps, core_ids=list(range(NCORES)))
    if any(np.isnan(np.asarray(res.results[c]["yq"], dtype=np.float32)).any()
           for c in range(NCORES)):
        # transient device-state glitch (seen once after an aborted compile):
        # one clean re-execution recovers
        res = run_bass_kernel_spmd(nc, in_maps, core_ids=list(range(NCORES)))
    _BASS_CACHE["last_results"] = res

    out = np.empty((b, 1, N, N), dtype=np.float32)
    for c in range(NCORES):
        ys = res.results[c]["yq"]
        for i in range(IPC):
            out[c * IPC + i, 0] = _unfold_image(ys[i])
    return out
